# revision 80
# baseline (speedup 1.0000x reference)
"""Trainium2 Bass kernel for the LSTM discriminator (fp8 DoubleRow version).

Model: LSTM(H=720) over x[B=2048, T=256, F=51], keep last hidden state,
then sigmoid -> Dense(1024) -> LeakyReLU(0.3) -> Dense(256) -> LeakyReLU(0.3)
-> Dense(1).

Strategy vs the bf16 baseline (which ran at the bf16 PE roofline, ~4.94ms):
  * Recurrence matmuls in fp8-e4m3 with MatmulPerfMode.DoubleRow: one
    instruction contracts TWO 128-row K-tiles (2x bf16 throughput on the PE).
    DoubleRow only pays off when the moving operand is wide, so z is computed
    *batch-major*: out[z^T] = [128 batch, 2880 gates], stationary = fp8
    h-state pairs [128, 2, 128], moving = fp8 Wh pairs [128, 2, <=512].
  * Quantization: Wh as e4m3(64*Wh), h as e4m3(16*h); the x projection stays
    bf16 (x-path quantization dominated the fp8 error budget) with weights
    pre-scaled by 1024 so both parts share one PSUM accumulation; the gate
    activation applies scale=1/1024. Measured end-to-end rel err ~3e-3.
  * x is pre-transposed on the host to [52, T, 128] bf16 per chain (51
    features + a ones-row that carries the LSTM bias), so the inner loop has
    no x transposes at all.
  * Two interleaved chains of 128 batch rows per core; z PSUM is 3 shared
    [128, 1024] tiles (6 banks) alternating between chains - ACT ops are
    tile-aligned so the write-after-read pacing is tile-granular.
  * Gate order in z columns is [i, f, o, g]: one sigmoid span (2160 cols) and
    one tanh span (720). Pointwise work is spread over ACT (gates + tanh(c)),
    GPSIMD (f*c, i*g) and DVE (c update, h=o*tanh(c), fp8 h-transpose copy).
  * h goes back to feature-major for the next step's stationary via 6 PE
    transposes (bf16, one PSUM bank), emitted mid-way through the *next*
    slot's matmul stream so the PE never waits on the pointwise chain.
  * fp32 head (sigmoid -> 3 dense layers), ~30us, after the loop.
"""

import os
import sys

import numpy as np

_TRN = "/opt/trn_rl_repo"
if _TRN not in sys.path:
    sys.path.insert(0, _TRN)

import ml_dtypes  # noqa: E402

import concourse.bacc as bacc  # noqa: E402
import concourse.tile as tile  # noqa: E402
from concourse import mybir  # noqa: E402
from concourse.bass_utils import run_bass_kernel_spmd  # noqa: E402

F32 = mybir.dt.float32
BF16 = mybir.dt.bfloat16
FP8 = mybir.dt.float8e4
AF = mybir.ActivationFunctionType
ALU = mybir.AluOpType
PM = mybir.MatmulPerfMode.DoubleRow

B, T_FULL, F, H = 2048, 256, 51, 720
D1, D2 = 1024, 256
NCORES = 8
BSH = B // NCORES   # 256 batch rows per core
NB = 128            # batch rows per chain (2 chains per core)
KX = F + 1          # x rows + ones row (bias)
G4 = 4 * H          # 2880
NP = 6              # h feature planes of 128 (720 -> 5x128 + 80, zero-padded)
TC = 32             # timesteps of x staged per DMA chunk
TPR = 4             # z-range index where the prev slot's h transposes go
S_W = 64.0          # Wh fp8 scale
S_H = 16.0          # h fp8 scale
S_Z = S_W * S_H     # total z scale (x-path weights pre-scaled by this)

# z column ranges (gate order g, i, f, o)
# tanh(g): cols [0, 720), sigmoid(i,f,o): cols [720, 2880)
# sg gate tile holds [i | f | o] at cols [0, 2160)
ZW = [512, 512, 512, 512, 512, 320]   # 6 ranges over 3 [128,1024] tiles

_NC_CACHE = {}
LAST_EXEC_NS = None
LAST_RESULTS = None


def _build(T):
    nc = bacc.Bacc(
        "TRN2", target_bir_lowering=False, debug=False, enable_asserts=False
    )

    xd = [
        nc.dram_tensor(f"x{c}", [KX, T, NB], BF16, kind="ExternalInput").ap()
        for c in range(2)
    ]
    whp_d = nc.dram_tensor("whp", [128, 3, 2, G4], FP8, kind="ExternalInput").ap()
    wxb_d = nc.dram_tensor("wxb", [KX, G4], BF16, kind="ExternalInput").ap()
    w1_d = nc.dram_tensor("w1", [128, NP, D1], F32, kind="ExternalInput").ap()
    w2_d = nc.dram_tensor("w2", [128, 8, D2], F32, kind="ExternalInput").ap()
    w3_d = nc.dram_tensor("w3", [128, 2, 1], F32, kind="ExternalInput").ap()
    b1_d = nc.dram_tensor("b1t", [128, 8], F32, kind="ExternalInput").ap()
    b2_d = nc.dram_tensor("b2t", [128, 2], F32, kind="ExternalInput").ap()
    b3_d = nc.dram_tensor("b3t", [1, 1], F32, kind="ExternalInput").ap()
    idb_d = nc.dram_tensor("identb", [128, 128], BF16, kind="ExternalInput").ap()
    idf_d = nc.dram_tensor("identf", [128, 128], F32, kind="ExternalInput").ap()
    out_d = nc.dram_tensor("out", [BSH, 1], F32, kind="ExternalOutput").ap()

    import contextlib

    with tile.TileContext(nc) as tc, contextlib.ExitStack() as ctx:
        if T > 16:
            tc.race_detector_enabled = False

        const = ctx.enter_context(tc.tile_pool(name="const", bufs=1))
        xpool = ctx.enter_context(tc.tile_pool(name="xp", bufs=2))
        gpool = ctx.enter_context(tc.tile_pool(name="gp", bufs=2))
        spool = ctx.enter_context(tc.tile_pool(name="st", bufs=1))
        zpool = ctx.enter_context(tc.tile_pool(name="zp", bufs=1, space="PSUM"))
        tpool = ctx.enter_context(tc.tile_pool(name="tp", bufs=1, space="PSUM"))

        # ---- constants / weights ----
        idb_t = const.tile([128, 128], BF16, tag="idb", name="idb")
        nc.sync.dma_start(idb_t[:], idb_d[:])
        idf_t = const.tile([128, 128], F32, tag="idf", name="idf")
        nc.sync.dma_start(idf_t[:], idf_d[:])
        whp_t = const.tile([128, 3, 2, G4], FP8, tag="whp", name="whp")
        wxb_t = const.tile([KX, G4], BF16, tag="wxb", name="wxb")
        w1_t = const.tile([128, NP, D1], F32, tag="w1", name="w1")
        w2_t = const.tile([128, 8, D2], F32, tag="w2", name="w2")
        w3_t = const.tile([128, 2, 1], F32, tag="w3", name="w3")
        b1_t = const.tile([128, 8], F32, tag="b1", name="b1")
        b2_t = const.tile([128, 2], F32, tag="b2", name="b2")
        b3_t = const.tile([1, 1], F32, tag="b3", name="b3")

        def load_weights():
            qs = [nc.gpsimd, nc.scalar, nc.sync]
            qi = [0]

            def dma(dst, src):
                qs[qi[0] % len(qs)].dma_start(dst, src)
                qi[0] += 1

            for k in range(NP):
                dma(w1_t[:, k, :], w1_d[:, k, :])
            for k in range(8):
                dma(w2_t[:, k, :], w2_d[:, k, :])
            dma(w3_t[:], w3_d[:])
            dma(b1_t[:], b1_d[:])
            dma(b2_t[:], b2_d[:])
            dma(b3_t[:], b3_d[:])
            dma(wxb_t[:], wxb_d[:])
            # whp last: the warmup matmuls read it, so they start right after
            # the weight DMAs and ramp the PE clock before the loop
            for p in range(3):
                dma(whp_t[:, p, :, :], whp_d[:, p, :, :])

        # ---- persistent state ----
        # hTq: fp8 16*h, feature-major, one tile per DR pair (tile-granular
        # deps: DR pair p only waits on its own copy)
        # halves split at h column 512 (HA = planes 0-3 = pairs 0,1; HB = pair 2)
        HA = 512
        HB = H - HA
        hTqp = [
            [spool.tile([128, 2, NB], FP8, tag=f"hTq{c}{p}", name=f"hTq{c}{p}") for p in range(3)]
            for c in range(2)
        ]
        cA = [spool.tile([NB, HA], BF16, tag=f"cA{c}", name=f"cA{c}") for c in range(2)]
        cB = [spool.tile([NB, HB], BF16, tag=f"cB{c}", name=f"cB{c}") for c in range(2)]
        # gates (f32, batch-major): sig = [i | f | o], tg = tanh(g)
        sig = [spool.tile([NB, 2160], BF16, tag=f"sg{c}", name=f"sg{c}") for c in range(2)]
        tg = [spool.tile([NB, H], BF16, tag=f"tg{c}", name=f"tg{c}") for c in range(2)]
        tclA = [spool.tile([NB, HA], BF16, tag=f"tcA{c}", name=f"tcA{c}") for c in range(2)]
        tclB = [spool.tile([NB, HB], BF16, tag=f"tcB{c}", name=f"tcB{c}") for c in range(2)]
        t1a = [spool.tile([NB, HA], BF16, tag=f"t1a{c}", name=f"t1a{c}") for c in range(2)]
        t1b = [spool.tile([NB, HB], BF16, tag=f"t1b{c}", name=f"t1b{c}") for c in range(2)]
        t2g = [spool.tile([NB, H], BF16, tag=f"t2{c}", name=f"t2{c}") for c in range(2)]
        # h, bf16, batch-major halves (scaled by 16 in the hTq copies)
        hbA = [spool.tile([NB, HA], BF16, tag=f"hbA{c}", name=f"hbA{c}") for c in range(2)]
        hbB = [spool.tile([NB, HB], BF16, tag=f"hbB{c}", name=f"hbB{c}") for c in range(2)]
        for c in range(2):
            for p in range(3):
                nc.vector.memset(hTqp[c][p][:].bitcast(F32), 0.0)
            nc.vector.memset(cA[c][:], 0.0)
            nc.vector.memset(cB[c][:], 0.0)

        # shared z PSUM: 3 tiles [128, 1024] = 6 banks, alternating chains
        # (PSUM dep tracking is effectively whole-tile, so the tile split IS
        # the pacing granularity between one slot's ACT reads and the next
        # slot's matmul writes)
        zt = [zpool.tile([NB, 1024], F32, tag=f"z{i}", name=f"z{i}") for i in range(3)]
        # bf16 h-transpose landing, ping-pong banks; rows >= 80 of plane 5
        # never written - zeroed once
        tpta = tpool.tile([128, 2, NB], BF16, tag="tpta", name="tpta")
        tptb = tpool.tile([128, 2, NB], BF16, tag="tptb", name="tptb")
        nc.vector.memset(tpta[:].bitcast(F32), 0.0)
        nc.vector.memset(tptb[:].bitcast(F32), 0.0)

        # ---- x chunks ----
        nchunks = (T + TC - 1) // TC
        xtiles = [[None] * nchunks for _ in range(2)]

        def ensure_chunk(c, ch):
            if ch >= nchunks or xtiles[c][ch] is not None:
                return
            sz = min(TC, T - ch * TC)
            t_ = xpool.tile([KX, TC, NB], BF16, tag=f"xc{c}", name=f"xc{c}")
            nc.sync.dma_start(t_[:, :sz, :], xd[c][:, ch * TC : ch * TC + sz, :])
            xtiles[c][ch] = t_

        ensure_chunk(0, 0)
        ensure_chunk(1, 0)
        load_weights()

        # ---- HAM warmup: PE busy ~5us right after the whp DMA lands ----
        for w_ in range(10):
            nc.tensor.matmul(
                zt[0][:, 0:512],
                hTqp[0][0][:],
                whp_t[:, 2, :, 2368:2880],
                start=True,
                stop=True,
                perf_mode=PM,
            )

        def emit_htranspose(s, pairs):
            # transpose bf16 h of slot s into ping-pong PSUM tiles, then
            # scale-cast (x16 -> fp8) into hTqp, per DR pair so pair p only
            # gates on its own copy
            c = s % 2
            for p in pairs:
                dst = (tpta, tptb, tpta)[p]
                for q in range(2):
                    j = 2 * p + q
                    col = 128 * j
                    w = min(128, H - col)
                    src = hbA[c] if col < HA else hbB[c]
                    off = col if col < HA else col - HA
                    nc.tensor.transpose(
                        dst[0:w, q, :], src[:, off : off + w], idb_t[:]
                    )
                nc.vector.tensor_scalar_mul(hTqp[c][p][:], dst[:], S_H)

        def emit_z(s):
            c, t = s % 2, s // 2
            ch, toff = t // TC, t % TC
            if toff == 0:
                ensure_chunk(c, ch + 1)
            xsl = xtiles[c][ch][:, toff, :]

            def zrange(r):
                return zt[r // 2][:, (r % 2) * 512 : (r % 2) * 512 + ZW[r]]

            # tile-grouped emission: finish each z tile's accumulation
            # completely (x + 3 DR passes over its two ranges) before moving
            # on, so the gate ACT ops start as early as possible and release
            # the tiles for the next slot in stream order
            cols = [0, 512, 1024, 1536, 2048, 2560]
            do_tp = s >= 1 and (s - 1) // 2 < T - 1
            for grp in ((0, 1, 2, 3), (5, 4)):
                for r in grp:
                    nc.tensor.matmul(
                        zrange(r), xsl, wxb_t[:, cols[r] : cols[r] + ZW[r]],
                        start=True, stop=False,
                    )
                for p in range(3):
                    for r in grp:
                        nc.tensor.matmul(
                            zrange(r),
                            hTqp[c][p][:],
                            whp_t[:, p, :, cols[r] : cols[r] + ZW[r]],
                            start=False,
                            stop=(p == 2),
                            perf_mode=PM,
                        )
                if do_tp and len(grp) == 4:
                    emit_htranspose(s - 1, (0, 1, 2))


        def emit_pointwise(s):
            c, t = s % 2, s // 2
            # gates: z tiles -> SBUF (tile-bounded ACT ops), descale 1/S_Z
            # z cols [g | i | f | o]; sg = [i | f | o]
            nc.scalar.activation(tg[c][:], zt[0][:, 0:720], AF.Tanh, scale=1.0 / S_Z)
            nc.scalar.activation(
                sig[c][:, 0:304], zt[0][:, 720:1024], AF.Sigmoid, scale=1.0 / S_Z
            )
            nc.scalar.activation(
                sig[c][:, 304:816], zt[1][:, 0:512], AF.Sigmoid, scale=1.0 / S_Z
            )
            nc.scalar.activation(
                sig[c][:, 816:1328], zt[1][:, 512:1024], AF.Sigmoid, scale=1.0 / S_Z
            )
            # sig3 split: f-tail first (unblocks t1b), then o
            nc.scalar.activation(
                sig[c][:, 1328:1440], zt[2][:, 0:112], AF.Sigmoid, scale=1.0 / S_Z
            )
            nc.scalar.activation(
                sig[c][:, 1440:2160], zt[2][:, 112:832], AF.Sigmoid, scale=1.0 / S_Z
            )
            # cell update halves at HA=512: t1 = f*c (gpsimd, t1a starts
            # early), t2 = i*g (DVE), adds (DVE), tanh(c) halves (ACT),
            # h*16 -> bf16 (DVE)
            nc.gpsimd.tensor_mul(t1a[c][:], sig[c][:, 720 : 720 + HA], cA[c][:])
            nc.gpsimd.tensor_mul(t1b[c][:], sig[c][:, 720 + HA : 1440], cB[c][:])
            nc.vector.tensor_mul(t2g[c][:, 0:HA], sig[c][:, 0:HA], tg[c][:, 0:HA])
            nc.vector.tensor_mul(t2g[c][:, HA:H], sig[c][:, HA:720], tg[c][:, HA:H])
            nc.vector.tensor_add(cA[c][:], t1a[c][:], t2g[c][:, 0:HA])
            nc.vector.tensor_add(cB[c][:], t1b[c][:], t2g[c][:, HA:H])
            nc.scalar.activation(tclA[c][:], cA[c][:], AF.Tanh)
            nc.scalar.activation(tclB[c][:], cB[c][:], AF.Tanh)
            if t < T - 1:
                nc.vector.tensor_mul(hbA[c][:], sig[c][:, 1440 : 1440 + HA], tclA[c][:])
                nc.vector.tensor_mul(hbB[c][:], sig[c][:, 1440 + HA : 2160], tclB[c][:])

        S = 2 * T
        for s in range(S):
            emit_z(s)
            emit_pointwise(s)

        # ---- head ----
        sgT = spool.tile([128, NP, BSH], F32, tag="sgT", name="sgT")
        hf = spool.tile([NB, H], F32, tag="hf", name="hf")
        for c in range(2):
            nc.vector.tensor_mul(hf[:, 0:HA], sig[c][:, 1440 : 1440 + HA], tclA[c][:])
            nc.vector.tensor_mul(hf[:, HA:H], sig[c][:, 1440 + HA : 2160], tclB[c][:])
            nc.scalar.activation(hf[:], hf[:], AF.Sigmoid)
            for j in range(NP):
                w = min(128, H - 128 * j)
                nc.tensor.transpose(
                    zt[0][0:w, 128 * j : 128 * j + 128],
                    hf[:, 128 * j : 128 * j + w],
                    idf_t[:],
                )
            nc.vector.tensor_copy(
                sgT[:, :, c * NB : c * NB + NB],
                zt[0][:, 0:768],
            )

        o1 = spool.tile([128, 8, BSH], F32, tag="o1", name="o1")
        for m in range(8):
            ps = zt[1][:, 0:BSH]
            for j in range(NP):
                nc.tensor.matmul(
                    ps,
                    w1_t[:, j, m * 128 : (m + 1) * 128],
                    sgT[:, j, :],
                    start=(j == 0),
                    stop=(j == NP - 1),
                )
            tb = gpool.tile([128, BSH], F32, tag="hb2", name="hb2")
            nc.vector.tensor_scalar_add(tb[:], ps, b1_t[:, m : m + 1])
            nc.vector.scalar_tensor_tensor(
                o1[:, m, :], tb[:], 0.3, tb[:], ALU.mult, ALU.max
            )
        o2 = spool.tile([128, 2, BSH], F32, tag="o2", name="o2")
        for m in range(2):
            ps = zt[2][:, 0:BSH]
            for k in range(8):
                nc.tensor.matmul(
                    ps,
                    w2_t[:, k, m * 128 : (m + 1) * 128],
                    o1[:, k, :],
                    start=(k == 0),
                    stop=(k == 7),
                )
            tb = gpool.tile([128, BSH], F32, tag="hb2", name="hb2")
            nc.vector.tensor_scalar_add(tb[:], ps, b2_t[:, m : m + 1])
            nc.vector.scalar_tensor_tensor(
                o2[:, m, :], tb[:], 0.3, tb[:], ALU.mult, ALU.max
            )
        ps = zt[1][0:1, 512 : 512 + BSH]
        for k in range(2):
            nc.tensor.matmul(
                ps, w3_t[:, k, :], o2[:, k, :], start=(k == 0), stop=(k == 1)
            )
        ob = spool.tile([1, BSH], F32, tag="ob", name="ob")
        nc.vector.tensor_scalar_add(ob[:], ps, b3_t[:])
        nc.sync.dma_start(out_d[:], ob[:])

    nc.compile()
    return nc


def _get_nc(T):
    if T not in _NC_CACHE:
        _NC_CACHE[T] = _build(T)
    return _NC_CACHE[T]


def _prep_weights(Wx, Wh, b, W1, b1, W2, b2, W3, b3):
    f32 = np.float32
    bf = ml_dtypes.bfloat16
    e4 = ml_dtypes.float8_e4m3fn

    # gate reorder: reference z = [i | f | g | o] -> ours [g | i | f | o]
    def reord(w):
        return np.concatenate(
            [w[..., 1440:2160], w[..., 0:1440], w[..., 2160:2880]], axis=-1
        )

    Whr = reord(np.asarray(Wh, f32))
    Wxr = reord(np.asarray(Wx, f32))
    br = reord(np.asarray(b, f32).reshape(1, G4))[0]

    whp = np.zeros((128, 3, 2, G4), f32)
    for p in range(3):
        for q in range(2):
            r0 = 128 * (2 * p + q)
            r1 = min(r0 + 128, H)
            if r0 < H:
                whp[0 : r1 - r0, p, q, :] = Whr[r0:r1]
    whp = np.clip(whp * S_W, -240, 240).astype(e4)

    wxb = np.zeros((KX, G4), f32)
    wxb[:F] = Wxr
    wxb[F] = br
    wxb = (wxb * S_Z).astype(bf)

    w1 = np.zeros((128, NP, D1), f32)
    W1a = np.asarray(W1, f32)
    for j in range(NP):
        r0 = 128 * j
        r1 = min(r0 + 128, H)
        w1[0 : r1 - r0, j, :] = W1a[r0:r1]
    w2 = np.ascontiguousarray(np.asarray(W2, f32).reshape(8, 128, D2).transpose(1, 0, 2))
    w3 = np.ascontiguousarray(np.asarray(W3, f32).reshape(2, 128, 1).transpose(1, 0, 2))
    b1t = np.ascontiguousarray(np.asarray(b1, f32).reshape(8, 128).T)
    b2t = np.ascontiguousarray(np.asarray(b2, f32).reshape(2, 128).T)
    b3t = np.asarray(b3, f32).reshape(1, 1)
    return {
        "whp": whp,
        "wxb": wxb,
        "w1": w1,
        "w2": w2,
        "w3": w3,
        "b1t": b1t,
        "b2t": b2t,
        "b3t": b3t,
        "identb": np.eye(128, dtype=bf),
        "identf": np.eye(128, dtype=f32),
    }


def kernel(x, Wx, Wh, b, W1, b1, W2, b2, W3, b3):
    global LAST_EXEC_NS, LAST_RESULTS
    x = np.asarray(x, dtype=np.float32)
    T = x.shape[1]
    nc = _get_nc(T)
    bf = ml_dtypes.bfloat16

    shared = _prep_weights(Wx, Wh, b, W1, b1, W2, b2, W3, b3)

    in_maps = []
    for i in range(NCORES):
        xs = x[i * BSH : (i + 1) * BSH]
        m = dict(shared)
        for c in range(2):
            chain = xs[c * NB : (c + 1) * NB]  # [128, T, 51]
            arr = np.empty((KX, T, NB), bf)
            arr[:F] = chain.transpose(2, 1, 0).astype(bf)
            arr[F] = np.float32(1.0)
            m[f"x{c}"] = arr
        in_maps.append(m)

    trace = bool(os.environ.get("KLSTM_TRACE"))
    res = run_bass_kernel_spmd(nc, in_maps, list(range(NCORES)), trace=trace)
    LAST_RESULTS = res
    LAST_EXEC_NS = res.exec_time_ns
    out = np.concatenate([r["out"] for r in res.results], axis=0)
    return out.astype(np.float32)
